# revision 53
# baseline (speedup 1.0000x reference)
"""Trainium2 Bass kernel for a noisy LSTMCell forward.

  gates = input @ W_ih.T + b_ih + hx @ W_hh.T + b_hh          # [B, 4H]
  i, f, g, o = split(gates); i,f,o=sigmoid, g=tanh
  cy = f*cx + i*g + sqrt(noise_e)*eps_c
  hy = o*tanh(cy) + sqrt(noise_q)*eps_h

B=4096, I=H=1024. Sharding: 2D grid over 8 NeuronCores — 4 batch shards
x 2 gate shards (minimizes per-core HBM traffic vs pure data-parallel).

Everything on device is kept feature-major ([feature, batch]) so the
matmul contraction dim lands on SBUF partitions with zero on-device
transposes and the ACT engine's per-partition bias applies the gate bias
directly during PSUM eviction. Matmul operands, the elementwise inputs
(cx/eps) and the hy/cy stores are fp16 (halves HBM traffic, full-rate
PE); gate accumulation (PSUM) and the elementwise chain stay fp32, and
the outputs are upcast to fp32 on the host. Overlap structure: warmup
matmuls ride out the PE clock ramp during the initial DMA fill, the
first three weight blocks are interleaved per k-tile so the PE never
starves while the activation stream lands, weights prefetch on a
6-deep rotation, and the last block runs chunk-major at 256 cols so the
eviction->elementwise->store tail pipelines off the critical path.
DMA transfers are packed to minimize instruction count (41/iteration):
k-tiles and weight blocks in pairs, the three elementwise inputs in one
per-h-tile transfer — each DMA instruction costs ~0.6us of HWDGE issue
plus a semaphore round trip on the consumer side.
Host-side prep (numpy): concat/transpose of activations, pre-tiling of
W so every DMA is one contiguous run per partition, sqrt of the noise
scalars.
Measured: ~98 us/iteration on 8 axon trn2 cores (PE-probe floor ~93 us,
DMA-probe ~43-63 us), rel err ~7.8e-3 vs the fp32 reference.
"""

import os
import sys
import numpy as np

for _p in ("/opt/trn_rl_repo", "/root/.axon_site/_ro/trn_rl_repo"):
    if _p not in sys.path and os.path.isdir(_p):
        sys.path.append(_p)

B, I, H = 4096, 1024, 1024
G = 4 * H                 # gate rows total
K = I + H                 # contraction dim
P_B, P_G = 4, 2           # batch shards x gate shards = 8 cores
BS = B // P_B             # 1024 batch cols per core
HS = H // P_G             # 512 h rows per core
NKT = K // 128            # 16 contraction tiles
NHT = HS // 128           # 4 h tiles per core
NA = NHT * 4              # 16 weight blocks (ht-major, gate-minor)
NBC = BS // 512           # 2 batch chunks of 512 (fp32 PSUM free-dim max)

MM_DT = os.environ.get("KM_MM_DT", "float16")   # matmul operand dtype on the PE
EW_DT = os.environ.get("KM_EW_DT", "float16")   # cx/eps DMA dtype

_LAST = None              # BassKernelResults of the most recent run (for test.py)


def _build_nc(mm_dt=None, ew_dt=None, iters=1, wbufs=4):
    import concourse.bacc as bacc
    import concourse.tile as tile
    from concourse import mybir
    from contextlib import ExitStack

    mm_dt = mm_dt or MM_DT
    ew_dt = ew_dt or EW_DT
    f32 = mybir.dt.float32
    mdt = getattr(mybir.dt, mm_dt)
    edt = getattr(mybir.dt, ew_dt)
    AF = mybir.ActivationFunctionType
    OP = mybir.AluOpType
    nc = bacc.Bacc("TRN2", target_bir_lowering=False)

    # DMA-instruction count is minimized by packing transfers: k-tiles in
    # pairs ([128, 2*BS], one 4KB descriptor per partition), weight blocks
    # in pairs ([128, 2*NKT*128], one 8KB descriptor), and cx/eps_c/eps_h
    # concatenated per h-tile row ([HS, 3*BS]). Each DMA instruction costs
    # ~0.6us of HWDGE issue plus a semaphore round trip, so fewer/larger
    # transfers shorten every dependence chain.
    xT = nc.declare_dram_parameter("xT", [NKT // 2, 128, 2 * BS], mdt,
                                   isOutput=False)
    w = nc.declare_dram_parameter("w", [NA // 2, 128, 2 * NKT * 128], mdt,
                                  isOutput=False)
    bias = nc.declare_dram_parameter("bias", [128, NA], f32, isOutput=False)
    ewc = nc.declare_dram_parameter("ewc", [HS, 3 * BS], edt, isOutput=False)
    noise = nc.declare_dram_parameter("noise", [2], f32, isOutput=False)
    # Outputs are stored at edt (fp16) and upcast to fp32 on the host in
    # _gather — halves the store traffic; the elementwise math stays fp32.
    hyT = nc.declare_dram_parameter("hyT", [HS, BS], edt, isOutput=True)
    cyT = nc.declare_dram_parameter("cyT", [HS, BS], edt, isOutput=True)

    with tile.TileContext(nc) as tc, ExitStack() as ctx:
        xpool = ctx.enter_context(tc.tile_pool(name="xpool", bufs=1))
        wpool = ctx.enter_context(tc.tile_pool(name="wpool", bufs=wbufs))
        psum = ctx.enter_context(tc.tile_pool(name="psum", bufs=4, space="PSUM"))
        gates = ctx.enter_context(tc.tile_pool(name="gates", bufs=2))
        ew = ctx.enter_context(tc.tile_pool(name="ew", bufs=2))
        const = ctx.enter_context(tc.tile_pool(name="const", bufs=1))

        for _it in range(iters):
            w_tiles = {}     # pair index -> [128, 2*NKT*128] tile

            def load_w(j):
                t = wpool.tile([128, 2 * NKT * 128], mdt, tag="w_t", name=f"w{j}")
                nc.sync.dma_start(out=t[:], in_=w[j, :, :])
                w_tiles[j] = t

            def wsl(a, kt):
                # lhsT slice for block a, contraction tile kt
                base = (a % 2) * (NKT * 128) + kt * 128
                return w_tiles[a // 2][:, base:base + 128]

            # Weight block 0 is DMA'd before the resident activations so the
            # PE can start as soon as xk[0] lands; block 1 is interleaved
            # mid-stream. Later blocks are requested just-in-time (wbufs
            # rotation gives the prefetch depth). The constants (bias,
            # noise scalars) aren't needed until the first PSUM eviction,
            # so their DMAs queue after the startup-critical ones.
            # Warmup: PE clock (HAM) needs ~3.4us of sustained matmul
            # activity to reach full rate, and the PE sits idle for about
            # that long waiting on the first weight/activation DMAs anyway.
            # Run throwaway matmuls on a zeroed tile so the real matmuls
            # start at full clock. PSUM slot is borrowed from the ps0 tag
            # (released before the first real accumulation needs it).
            warm_src = xpool.tile([128, 512], mdt, tag="warm", name="warm_src")
            nc.vector.memset(warm_src[:], 0.0)
            warm_ps = psum.tile([128, 512], f32, tag="ps0", name="warm_ps")
            for _wi in range(10):
                nc.tensor.matmul(warm_ps[:], warm_src[:, 0:128], warm_src[:],
                                 start=True, stop=True)

            # DMA queue order tuned for the fill phase: the first two xk
            # tiles, then the first two weight blocks, then the rest of the
            # xk stream. (The cost-model DMA device drains transfers in
            # queue order; this ordering has the first matmul's operands
            # land just as the warmup finishes.)
            xkp = []

            def load_xk(i):
                t = xpool.tile([128, 2 * BS], mdt, tag=f"xk{i}", name=f"xk{i}")
                nc.sync.dma_start(out=t[:], in_=xT[i, :, :])
                xkp.append(t)

            def xsl(kt, lo, hi):
                base = (kt % 2) * BS
                return xkp[kt // 2][:, base + lo:base + hi]

            load_xk(0)           # k-tiles 0,1
            load_w(0)            # blocks 0,1
            load_w(1)            # blocks 2,3
            for i in range(1, NKT // 2):
                load_xk(i)

            # noise holds [sqrt(noise_e), sqrt(noise_q)] — the sqrt is done
            # in host prep so the ACT engine needs only the sigmoid/tanh
            # function table (no sqrt-table switch).
            bias_t = const.tile([128, NA], f32, tag="bias")
            nc.sync.dma_start(out=bias_t[:], in_=bias[:, :])
            se_t = const.tile([128, 1], f32, tag="se")   # sqrt(noise_e)
            sq_t = const.tile([128, 1], f32, tag="sq")   # sqrt(noise_q)
            nc.sync.dma_start(out=se_t[:], in_=noise[0:1].to_broadcast([128, 1]))
            nc.sync.dma_start(out=sq_t[:], in_=noise[1:2].to_broadcast([128, 1]))

            def emit_mms_kt_major(a, ps):
                for kt in range(NKT):
                    for c in range(NBC):
                        nc.tensor.matmul(
                            ps[c][:], wsl(a, kt), xsl(kt, c * 512, (c + 1) * 512),
                            start=(kt == 0), stop=(kt == NKT - 1),
                        )

            def evict(gate_tile, ps, a, func):
                for c in range(NBC):
                    nc.scalar.activation(
                        gate_tile[:, c * 512:(c + 1) * 512],
                        ps[c][:],
                        func,
                        bias=bias_t[:, a:a + 1],
                    )

            for ht in range(NHT):
                last_ht = ht == NHT - 1
                # Prefetch this h-tile's weight pairs plus one pair ahead.
                for j in (2 * ht, 2 * ht + 1, 2 * ht + 2):
                    if j < NA // 2 and j not in w_tiles:
                        load_w(j)
                gt = [gates.tile([128, BS], f32, tag=f"g{gate}", name=f"gt{gate}") for gate in range(4)]
                i_t, f_t, g_t, o_t = gt
                row = slice(ht * 128, (ht + 1) * 128)
                ewt = ew.tile([128, 3 * BS], edt, tag="ewt", name="ewt")
                nc.sync.dma_start(out=ewt[:], in_=ewc[row, :])

                def cx_s(lo, hi):
                    return ewt[:, lo:hi]

                def ec_s(lo, hi):
                    return ewt[:, BS + lo:BS + hi]

                def eh_s(lo, hi):
                    return ewt[:, 2 * BS + lo:2 * BS + hi]

                if ht == 0:
                    # Gates 0-2 interleaved per k-tile: while the xk stream
                    # is still arriving from HBM, each landed pair feeds 12
                    # matmuls, so the PE never starves even under bursty
                    # DMA and the clock ramp is not reset during the fill.
                    pss = [[psum.tile([128, 512], f32, tag=f"ps{c}",
                                      name=f"ps{g}{c}") for c in range(NBC)]
                           for g in range(3)]
                    for kt in range(NKT):
                        for g in range(3):
                            for c in range(NBC):
                                nc.tensor.matmul(
                                    pss[g][c][:], wsl(g, kt),
                                    xsl(kt, c * 512, (c + 1) * 512),
                                    start=(kt == 0), stop=(kt == NKT - 1),
                                )
                    evict(i_t, pss[0], 0, AF.Sigmoid)
                    evict(f_t, pss[1], 1, AF.Sigmoid)
                    evict(g_t, pss[2], 2, AF.Tanh)
                    first_gate = 3
                else:
                    first_gate = 0

                for gate in range(first_gate, 3):
                    a = ht * 4 + gate
                    if a // 2 not in w_tiles:
                        load_w(a // 2)
                    ps = [psum.tile([128, 512], f32, tag=f"ps{c}", name=f"ps{c}") for c in range(NBC)]
                    emit_mms_kt_major(a, ps)
                    evict(gt[gate], ps, a, AF.Tanh if gate == 2 else AF.Sigmoid)

                # cy chain (does not depend on the outgate): cyo becomes cy
                # (fp16 for the store; the adds happen in fp32), g_t becomes
                # tanh(cy). Chunked at 512 so chunk-0 stores overlap
                # chunk-1 compute.
                cyo = ew.tile([128, BS], edt, tag="cyo", name="cyo")
                hyo = ew.tile([128, BS], edt, tag="hyo", name="hyo")
                for c in range(NBC):
                    cs = slice(c * 512, (c + 1) * 512)
                    nc.vector.tensor_mul(f_t[:, cs], f_t[:, cs],
                                         cx_s(cs.start, cs.stop))
                    nc.vector.tensor_mul(i_t[:, cs], i_t[:, cs], g_t[:, cs])
                    nc.vector.tensor_add(f_t[:, cs], f_t[:, cs], i_t[:, cs])
                    nc.vector.scalar_tensor_tensor(              # + se*ec = cy
                        cyo[:, cs], ec_s(cs.start, cs.stop), se_t[:, 0:1],
                        f_t[:, cs],
                        OP.mult, OP.add,
                    )
                    nc.scalar.activation(g_t[:, cs], cyo[:, cs], AF.Tanh)
                    nc.sync.dma_start(out=cyT[row, cs], in_=cyo[:, cs])

                if not last_ht:
                    # hy chain at 512 chunks
                    a = ht * 4 + 3
                    if a // 2 not in w_tiles:
                        load_w(a // 2)
                    ps = [psum.tile([128, 512], f32, tag=f"ps{c}", name=f"ps{c}") for c in range(NBC)]
                    emit_mms_kt_major(a, ps)
                    for c in range(NBC):
                        cs = slice(c * 512, (c + 1) * 512)
                        nc.scalar.activation(
                            o_t[:, cs], ps[c][:], AF.Sigmoid,
                            bias=bias_t[:, a:a + 1],
                        )
                        nc.vector.tensor_mul(o_t[:, cs], o_t[:, cs], g_t[:, cs])
                        nc.vector.scalar_tensor_tensor(          # + sq*eh = hy
                            hyo[:, cs], eh_s(cs.start, cs.stop), sq_t[:, 0:1],
                            o_t[:, cs],
                            OP.mult, OP.add,
                        )
                        nc.sync.dma_start(out=hyT[row, cs], in_=hyo[:, cs])
                else:
                    # Final block: sub-chunk-major at 256 cols so the tail
                    # (eviction -> o*tanh(cy) -> +sq*eh -> store) pipelines
                    # in four 256-wide waves while later waves are still on
                    # the PE. Costs extra LDWEIGHTS passes; trims the tail.
                    a = ht * 4 + 3
                    if a // 2 not in w_tiles:
                        load_w(a // 2)
                    for sc in range(4):
                        pss = psum.tile([128, 256], f32, tag=f"ps{sc % 2}",
                                        name=f"pss{sc}")
                        for kt in range(NKT):
                            nc.tensor.matmul(
                                pss[:],
                                wsl(a, kt),
                                xsl(kt, sc * 256, (sc + 1) * 256),
                                start=(kt == 0), stop=(kt == NKT - 1),
                            )
                        ss = slice(sc * 256, (sc + 1) * 256)
                        nc.scalar.activation(
                            o_t[:, ss], pss[:], AF.Sigmoid,
                            bias=bias_t[:, a:a + 1],
                        )
                        nc.vector.tensor_mul(o_t[:, ss], o_t[:, ss], g_t[:, ss])
                        nc.vector.scalar_tensor_tensor(          # + sq*eh = hy
                            hyo[:, ss], eh_s(ss.start, ss.stop), sq_t[:, 0:1],
                            o_t[:, ss],
                            OP.mult, OP.add,
                        )
                        nc.sync.dma_start(out=hyT[row, ss], in_=hyo[:, ss])

    nc.compile()
    return nc


def _np_dtype(dt_name):
    if dt_name in ("float32r", "float32"):
        return np.float32
    if dt_name == "float16":
        return np.float16
    if dt_name == "bfloat16":
        import ml_dtypes
        return ml_dtypes.bfloat16
    raise ValueError(dt_name)


def _prep_inputs(input, hx, cx, noise_q, noise_e,
                 weight_ih, weight_hh, bias_ih, bias_hh, eps_c, eps_h,
                 mm_dt=None, ew_dt=None):
    mdt = _np_dtype(mm_dt or MM_DT)
    edt = _np_dtype(ew_dt or EW_DT)
    f = lambda a: np.ascontiguousarray(np.asarray(a, dtype=np.float32))
    X = np.concatenate([f(input), f(hx)], axis=1)          # [B, K]
    XT = np.ascontiguousarray(X.T.astype(mdt))              # [K, B]
    W_cat = np.concatenate([f(weight_ih), f(weight_hh)], axis=1).astype(mdt)  # [G, K]
    bias_full = f(bias_ih) + f(bias_hh)                     # [G]
    cxT = f(cx).T.astype(edt)
    epcT = f(eps_c).T.astype(edt)
    ephT = f(eps_h).T.astype(edt)
    noise = np.sqrt(np.array([np.asarray(noise_e).reshape(-1)[0],
                              np.asarray(noise_q).reshape(-1)[0]],
                             dtype=np.float32))

    # Per gate-shard j: weight blocks in the exact consumption order
    # (a = ht*4 + gate), each pre-transposed to [k_p, kt*128 + g_c] so the
    # per-partition DMA stride is a single contiguous run; adjacent blocks
    # are packed in pairs for one 8KB-per-partition descriptor per DMA.
    w_host, bias_host = [], []
    for j in range(P_G):
        blocks, bcols = [], []
        for ht in range(NHT):
            for gate in range(4):
                g0 = gate * H + j * HS + ht * 128
                blk = W_cat[g0:g0 + 128, :]                        # (g_c, k)
                blocks.append(blk.reshape(128, NKT, 128).transpose(2, 1, 0)
                              .reshape(128, NKT * 128))
                bcols.append(bias_full[g0:g0 + 128])
        w_host.append(np.ascontiguousarray(np.stack(
            [np.concatenate([blocks[2 * p], blocks[2 * p + 1]], axis=1)
             for p in range(NA // 2)])))                   # [NA/2, 128, 2*NKT*128]
        bias_host.append(np.ascontiguousarray(np.stack(bcols, axis=1)))

    in_maps = []
    for bi in range(P_B):
        bcol = slice(bi * BS, (bi + 1) * BS)
        xP = XT[:, bcol]                                   # [K, BS]
        xpk = np.ascontiguousarray(np.stack(
            [np.concatenate([xP[(2 * i) * 128:(2 * i + 1) * 128, :],
                             xP[(2 * i + 1) * 128:(2 * i + 2) * 128, :]],
                            axis=1)
             for i in range(NKT // 2)]))                   # [NKT/2, 128, 2*BS]
        for j in range(P_G):
            hrow = slice(j * HS, (j + 1) * HS)
            ew_pack = np.ascontiguousarray(np.concatenate(
                [cxT[hrow, bcol], epcT[hrow, bcol], ephT[hrow, bcol]],
                axis=1))                                   # [HS, 3*BS]
            in_maps.append({
                "xT": xpk,
                "w": w_host[j],
                "bias": bias_host[j],
                "ewc": ew_pack,
                "noise": noise,
            })
    return in_maps


def _gather(results):
    hyT = np.empty((H, B), dtype=np.float32)
    cyT = np.empty((H, B), dtype=np.float32)
    idx = 0
    for bi in range(P_B):
        bcol = slice(bi * BS, (bi + 1) * BS)
        for j in range(P_G):
            hrow = slice(j * HS, (j + 1) * HS)
            hyT[hrow, bcol] = results[idx]["hyT"].astype(np.float32)
            cyT[hrow, bcol] = results[idx]["cyT"].astype(np.float32)
            idx += 1
    return np.ascontiguousarray(hyT.T), np.ascontiguousarray(cyT.T)


def kernel(**inputs):
    global _LAST
    from concourse.bass_utils import run_bass_kernel_spmd

    in_maps = _prep_inputs(**inputs)
    nc = _build_nc()
    _LAST = run_bass_kernel_spmd(nc, in_maps, list(range(8)), trace=False)
    return _gather(_LAST.results)


# ---------------------------------------------------------------------------
# Timing helper for test.py (not used by the grading path).
#
# The axon tunnel's dispatch round trip (~80ms) hides any single on-device
# execution, and its per-dispatch service throughput (~0.5ms) floors naive
# chained-dispatch slopes. To isolate hardware time: replicate the whole
# computation N times inside one NEFF (iterations serialize through SBUF
# slot reuse), pipeline B async dispatches of that NEFF back-to-back, and
# take the marginal per-dispatch wall. With N*T_hw well above the RPC
# service floor the pipeline is exec-bound, so
#     T_hw ≈ (slope_N − slope_1) / (N − 1)
# where slope_1 (the 1-iteration NEFF's marginal wall) measures the RPC
# service floor itself.
# ---------------------------------------------------------------------------

def benchmark(inputs, n_iter=33, reps=19, batch=32):
    import jax
    in_maps = _prep_inputs(**inputs)
    fn1, args1, out_names, out_avals = _make_exec(_build_nc(iters=1), in_maps)
    fnN, argsN, _, _ = _make_exec(_build_nc(iters=n_iter), in_maps)
    out1 = fn1(*args1)                      # compile + warm + result
    jax.block_until_ready(out1)
    jax.block_until_ready(fnN(*argsN))

    s1 = _pipelined_slope(fn1, args1, reps, batch)
    sN = _pipelined_slope(fnN, argsN, reps, batch)
    print(f"    slope1 (ms): {[f'{x*1e3:.3f}' for x in s1]}", flush=True)
    print(f"    slope{n_iter} (ms): {[f'{x*1e3:.3f}' for x in sN]}", flush=True)
    per_exec_ns = (sN[0] - s1[0]) / (n_iter - 1) * 1e9
    direct_ns = sN[0] / n_iter * 1e9
    print(f"    per-iter: marginal={per_exec_ns:.0f} ns, "
          f"direct(incl. RPC share)={direct_ns:.0f} ns", flush=True)

    n_cores = 8
    results = [
        {name: np.asarray(out1[i]).reshape(n_cores, *out_avals[i].shape)[c]
         for i, name in enumerate(out_names)}
        for c in range(n_cores)
    ]
    return per_exec_ns, s1[0] * 1e9, _gather(results)


def _make_exec(nc, in_maps):
    import jax
    from jax.sharding import Mesh, PartitionSpec, NamedSharding
    from jax.experimental.shard_map import shard_map
    from concourse import bass2jax, mybir
    from concourse.bass2jax import _bass_exec_p

    bass2jax.install_neuronx_cc_hook()
    assert nc.dbg_addr is None
    partition_name = nc.partition_id_tensor.name if nc.partition_id_tensor else None

    in_names, out_names, out_avals, zero_outs = [], [], [], []
    for alloc in nc.m.functions[0].allocations:
        if not isinstance(alloc, mybir.MemoryLocationSet):
            continue
        name = alloc.memorylocations[0].name
        if alloc.kind == "ExternalInput":
            if name != partition_name:
                in_names.append(name)
        elif alloc.kind == "ExternalOutput":
            shape = tuple(alloc.tensor_shape)
            dtype = mybir.dt.np(alloc.dtype)
            out_names.append(name)
            out_avals.append(jax.core.ShapedArray(shape, dtype))
            zero_outs.append(np.zeros(shape, dtype))
    n_params = len(in_names)
    all_in_names = tuple(in_names + out_names
                         + ([partition_name] if partition_name else []))

    def _body(*args):
        ins = list(args[:n_params])
        outs = tuple(args[n_params:])
        pid = [bass2jax.partition_id_tensor()] if partition_name else []
        return tuple(_bass_exec_p.bind(
            *ins, *outs, *pid,
            out_avals=tuple(out_avals),
            in_names=all_in_names,
            out_names=tuple(out_names),
            lowering_input_output_aliases=(),
            sim_require_finite=True,
            sim_require_nnan=True,
            nc=nc,
        ))

    n_cores = 8
    devices = jax.devices()[:n_cores]
    mesh = Mesh(np.asarray(devices), ("core",))
    spec = NamedSharding(mesh, PartitionSpec("core"))
    in_specs = (PartitionSpec("core"),) * (n_params + len(out_names))
    out_specs = (PartitionSpec("core"),) * len(out_names)

    concat_in = [
        np.concatenate([np.asarray(in_maps[c][name]) for c in range(n_cores)], axis=0)
        for name in in_names
    ]
    concat_zeros = [
        np.zeros((n_cores * z.shape[0], *z.shape[1:]), z.dtype) for z in zero_outs
    ]
    dev_args = [jax.device_put(a, spec) for a in concat_in + concat_zeros]
    jax.block_until_ready(dev_args)

    fn = jax.jit(shard_map(_body, mesh=mesh, in_specs=in_specs,
                           out_specs=out_specs, check_rep=False),
                 keep_unused=True)
    return fn, dev_args, out_names, out_avals


def _pipelined_slope(fn, dev_args, reps, batch):
    """Marginal per-dispatch wall of an async-pipelined dispatch stream,
    from the difference of a short and a long batch (the ~80ms RPC round
    trip cancels; only the per-dispatch service/exec slope remains)."""
    import time
    import jax

    b_small, b_large = 4, batch

    def timed(b):
        t0 = time.perf_counter()
        out = None
        for _i in range(b):
            out = fn(*dev_args)          # async dispatches queue in order
        jax.block_until_ready(out)
        return time.perf_counter() - t0

    ws, wl = [], []
    for _ in range(reps):
        ws.append(timed(b_small))
        wl.append(timed(b_large))
    # Wall-clock noise on this path is one-sided (late completions), so the
    # min over reps of each batch wall is the cleanest sample; slope from
    # the two mins avoids per-rep slope deflation when the small batch of
    # a pair happens to run late.
    ws.sort(); wl.sort()
    print(f"      walls b={b_small}: {[f'{x*1e3:.1f}' for x in ws[:5]]} ms | "
          f"b={b_large}: {[f'{x*1e3:.1f}' for x in wl[:5]]} ms", flush=True)
    slope = (wl[0] - ws[0]) / (b_large - b_small)
    return [slope] + [(l - s) / (b_large - b_small)
                      for l, s in zip(wl[1:], ws[1:])]
